# revision 17
# baseline (speedup 1.0000x reference)
"""Trainium2 Bass kernel for EpisodicMemory (DMN episodic memory module).

Full shapes: facts (128,256,512), questions/prevM (128,1,512), output (128,1,512).
Sharding: data-parallel over batch, 16 batches per core x 8 cores, weights
replicated. Activations kept feature-on-partition ("transposed") so matmuls
contract over the partition dim and pointwise ops run 128 lanes wide.

Per-core pipeline (v2 — bf16 everywhere on the PE so FWL fast-weight-load
kicks in; the fp32 path disables FWL and double-passes every matmul):
  P1  facts -> bf16 -> PE transposes -> fT_all (128, hc, b, s) resident
  P2  interaction features zT (bf16) -> z1 MLP (tanh) -> z2 logits
  P3  prall/phall = fT*rowsum(W) + bias  (bulk precompute, bf16)
  P4  softmax over S -> G broadcast gbc + (1-G) omgbc
  P5  GRU scan, 256 steps: PSUM seeded with prall[s]/bu via identity
      matmuls, bf16 weight-stationary matmuls, short DVE/ACT tail
  P6  next_mem = relu([prevM C q] @ nm_w + nm_b)
"""

from contextlib import ExitStack

import numpy as np

import concourse.bass as bass
import concourse.tile as tile
from concourse import bacc, masks, mybir
from concourse.bass_utils import run_bass_kernel_spmd

F32 = mybir.dt.float32
F32R = mybir.dt.float32r
BF16 = mybir.dt.bfloat16
AF = mybir.ActivationFunctionType
ALU = mybir.AluOpType

B, S, H = 128, 256, 512
N_CORES = 8
B_LOC = B // N_CORES  # 16


def build_nc(b_loc=B_LOC, s_len=S):
    """Build the per-core Bass program (SPMD: same program, sharded data)."""
    h = H
    nc = bacc.Bacc(
        "TRN2", target_bir_lowering=False, debug=False, num_devices=N_CORES
    )

    io = {}
    io["facts"] = nc.dram_tensor("facts", [b_loc, s_len, h], F32, kind="ExternalInput")
    io["questions"] = nc.dram_tensor("questions", [b_loc, 1, h], F32, kind="ExternalInput")
    io["prevM"] = nc.dram_tensor("prevM", [b_loc, 1, h], F32, kind="ExternalInput")
    io["z1_w"] = nc.dram_tensor("z1_w", [4 * h, h], F32, kind="ExternalInput")
    io["z1_b"] = nc.dram_tensor("z1_b", [h], F32, kind="ExternalInput")
    io["z2_w"] = nc.dram_tensor("z2_w", [h, 1], F32, kind="ExternalInput")
    for nm in ["Wr", "Ur", "W", "U"]:
        io[nm] = nc.dram_tensor(nm, [h, h], F32, kind="ExternalInput")
    for nm in ["br", "bur", "bw", "bu"]:
        io[nm] = nc.dram_tensor(nm, [h], F32, kind="ExternalInput")
    io["nm_w"] = nc.dram_tensor("nm_w", [3 * h, h], F32, kind="ExternalInput")
    io["nm_b"] = nc.dram_tensor("nm_b", [h], F32, kind="ExternalInput")
    io["out"] = nc.dram_tensor("out", [b_loc, 1, h], F32, kind="ExternalOutput")
    io["g_bounce"] = nc.dram_tensor("g_bounce", [s_len, b_loc], F32)
    io["logit_dram"] = nc.dram_tensor("logit_dram", [b_loc, s_len], F32)

    with tile.TileContext(nc) as tc:
        _body(tc, io, b_loc, s_len, h)
    nc.compile()
    return nc


def _body(tc, io, b_loc, s_len, h):
    nc = tc.nc
    hc = h // 128          # 4 h-chunks
    zc = 4 * hc            # 16 chunks of the 4H interaction dim
    gb = 2                 # batches per group for the z1 MLP
    ng = b_loc // gb
    sc_ = s_len // 128

    facts, questions, prevM = io["facts"], io["questions"], io["prevM"]

    with ExitStack() as ctx:
        # ---------------- resident pools ----------------
        wpool = ctx.enter_context(tc.tile_pool(name="wres", bufs=1))
        prepool = ctx.enter_context(tc.tile_pool(name="prepool", bufs=1))
        smallpool = ctx.enter_context(tc.tile_pool(name="small", bufs=1))

        # scan gate weights [Ur | U] in bf16: k-chunk c at cols [c*2h, (c+1)*2h)
        wcomb = wpool.tile([128, hc * 2 * h], BF16, tag="wcomb")
        wstg_pool = ctx.enter_context(tc.tile_pool(name="wstg", bufs=2))
        for c in range(hc):
            for gi, wn in enumerate(["Ur", "U"]):
                stg = wstg_pool.tile([128, h], F32, tag="wstg")
                nc.sync.dma_start(stg[:, :], io[wn][c * 128:(c + 1) * 128, :])
                nc.vector.tensor_copy(
                    wcomb[:, c * 2 * h + gi * h: c * 2 * h + (gi + 1) * h],
                    stg[:, :],
                )

        # small constants: (128, hc) with col = h-chunk
        def load_cvec(nm):
            t = smallpool.tile([128, hc], F32, tag=f"cv_{nm}")
            nc.sync.dma_start(t[:, :], io[nm].rearrange("(c p) -> p c", p=128))
            return t

        z1b4 = load_cvec("z1_b")
        br4 = load_cvec("br")
        bur4 = load_cvec("bur")
        bw4 = load_cvec("bw")
        bu4 = load_cvec("bu")
        z2c = smallpool.tile([128, hc], BF16, tag="z2c")
        z2stg = smallpool.tile([128, hc], F32, tag="z2stg")
        nc.sync.dma_start(
            z2stg[:, :], io["z2_w"].rearrange("(c p) o -> p (c o)", p=128)
        )
        nc.vector.tensor_copy(z2c[:, :], z2stg[:, :])
        brc4 = smallpool.tile([128, hc], F32, tag="brc4")  # br + bur
        nc.vector.tensor_copy(brc4[:, :], br4[:, :])
        nc.vector.tensor_add(brc4[:, :], brc4[:, :], bur4[:, :])

        ones_pb = smallpool.tile([128, b_loc], F32, tag="ones_pb")
        nc.vector.memset(ones_pb[:, :], 1.0)
        # psu PSUM seed: bu broadcast along batches, (128, hc, b_loc)
        bu_bc = smallpool.tile([128, hc, b_loc], BF16, tag="bu_bc")
        for c in range(hc):
            nc.vector.tensor_scalar_mul(bu_bc[:, c, :], ones_pb[:, :], bu4[:, c:c + 1])

        # questions / prevM transposed: (128, hc, b_loc)
        qT = smallpool.tile([128, hc, b_loc], F32R, tag="qT")
        mT = smallpool.tile([128, hc, b_loc], F32R, tag="mT")
        for bi in range(b_loc):
            nc.sync.dma_start(
                qT[:, :, bi],
                questions[bi, 0, :].rearrange("(c p) -> p c", p=128).bitcast(F32R),
            )
            nc.sync.dma_start(
                mT[:, :, bi],
                prevM[bi, 0, :].rearrange("(c p) -> p c", p=128).bitcast(F32R),
            )
        nqT = smallpool.tile([128, hc, b_loc], F32, tag="nqT")
        nmT = smallpool.tile([128, hc, b_loc], F32, tag="nmT")
        nc.vector.tensor_scalar_mul(nqT[:, :, :], qT[:, :, :].bitcast(F32), -1.0)
        nc.vector.tensor_scalar_mul(nmT[:, :, :], mT[:, :, :].bitcast(F32), -1.0)

        ones_row = smallpool.tile([1, b_loc], F32R, tag="ones_row")
        ones_stg = smallpool.tile([1, b_loc], F32, tag="ones_stg")
        nc.vector.memset(ones_stg[:, :], 1.0)
        nc.vector.tensor_copy(ones_row[:, :], ones_stg[:, :])
        nmb_row = smallpool.tile([1, h], F32R, tag="nmb_row")
        nc.sync.dma_start(nmb_row[:, :], io["nm_b"][None, :].bitcast(F32R))

        identb = smallpool.tile([128, 128], BF16, tag="identb")
        masks.make_identity(nc, identb[:, :])

        # facts transposed, resident: (128, hc, b_loc, s_len) bf16
        fT_all = prepool.tile([128, hc, b_loc, s_len], BF16, tag="fT_all")
        # pre-activations incl. biases, resident: (128, hc, b_loc, s_len) bf16
        prall = prepool.tile([128, hc, b_loc, s_len], BF16, tag="prall")
        phall = prepool.tile([128, hc, b_loc, s_len], BF16, tag="phall")
        logit = smallpool.tile([b_loc, s_len], F32, tag="logit")

        # row-sums of Wr / W (the reference's einsum "bsh,hk->bsh"
        # multiplies facts elementwise by these row-sums)
        rs4 = smallpool.tile([128, hc, 2], F32, tag="rs4")

        # ============ P1: load facts, cast bf16, PE-transpose ============
        with (
            tc.tile_pool(name="ph", bufs=3) as ph,
            tc.tile_pool(name="tps", bufs=2, space="PSUM") as tps,
        ):
            for gate, wname in enumerate(["Wr", "W"]):
                for c in range(hc):
                    wstg = ph.tile([128, h], F32, tag="rstg")
                    nc.sync.dma_start(
                        wstg[:, :], io[wname][c * 128:(c + 1) * 128, :]
                    )
                    nc.vector.tensor_reduce(
                        rs4[:, c, gate:gate + 1], wstg[:, :],
                        mybir.AxisListType.X, ALU.add,
                    )

            for bi in range(b_loc):
                fnat = ph.tile([128, sc_, h], F32, tag="fnat")
                nc.sync.dma_start(
                    fnat[:, :, :],
                    facts[bi].rearrange("(c p) h -> p c h", p=128),
                )
                fnb = ph.tile([128, sc_, h], BF16, tag="fnb")
                nc.scalar.copy(fnb[:, :, :], fnat[:, :, :])
                for sh in range(sc_):
                    tp = tps.tile([128, hc, 128], BF16, tag="tpsum")
                    for c in range(hc):
                        nc.tensor.transpose(
                            tp[:, c, :], fnb[:, sh, c * 128:(c + 1) * 128],
                            identb[:, :],
                        )
                    nc.vector.tensor_copy(
                        fT_all[:, :, bi, sh * 128:(sh + 1) * 128], tp[:, :, :]
                    )

        # ============ P2: interaction features + z1 MLP + z2 logits ====
        with (
            tc.tile_pool(name="z1wp", bufs=1) as z1wp,
            tc.tile_pool(name="ph2", bufs=2) as ph2,
            tc.tile_pool(name="zpool", bufs=3) as zp,
            tc.tile_pool(name="ghps", bufs=1, space="PSUM") as ghps,
            tc.tile_pool(name="lgps", bufs=1, space="PSUM") as lgps,
        ):
            # z1 weights in bf16 (staged through f32)
            z1w = z1wp.tile([128, zc * h], BF16, tag="z1w")
            for k in range(zc):
                stg = ph2.tile([128, h], F32, tag="z1stage")
                nc.sync.dma_start(stg[:, :], io["z1_w"][k * 128:(k + 1) * 128, :])
                nc.vector.tensor_copy(z1w[:, k * h:(k + 1) * h], stg[:, :])

            for g in range(ng):
                ghp = [ghps.tile([128, gb * s_len], F32, name=f"ghp{m}",
                                 tag=f"ghp{m}") for m in range(hc)]
                for k in range(zc):
                    zk = zp.tile([128, gb * s_len], BF16, tag="zk")
                    kind, c = divmod(k, hc)  # 0:f*q 1:f*m 2:|f-q| 3:|f-m|
                    for bp in range(gb):
                        bi = g * gb + bp
                        dst = zk[:, bp * s_len:(bp + 1) * s_len]
                        src = fT_all[:, c, bi, :]
                        if kind == 0:
                            nc.vector.tensor_scalar_mul(dst, src, qT[:, c, bi:bi + 1].bitcast(F32))
                        elif kind == 1:
                            nc.vector.tensor_scalar_mul(dst, src, mT[:, c, bi:bi + 1].bitcast(F32))
                        elif kind == 2:
                            nc.scalar.activation(dst, src, AF.Abs,
                                                 bias=nqT[:, c, bi:bi + 1])
                        else:
                            nc.scalar.activation(dst, src, AF.Abs,
                                                 bias=nmT[:, c, bi:bi + 1])
                    for m in range(hc):
                        nc.tensor.matmul(
                            ghp[m][:, :],
                            z1w[:, k * h + m * 128: k * h + (m + 1) * 128],
                            zk[:, :],
                            start=(k == 0),
                            stop=(k == zc - 1),
                        )
                ghT = ph2.tile([128, hc, gb * s_len], BF16, tag="ghT")
                for m in range(hc):
                    nc.scalar.activation(
                        ghT[:, m, :], ghp[m][:, :], AF.Tanh,
                        bias=z1b4[:, m:m + 1],
                    )
                lgp = lgps.tile([1, gb * s_len], F32, tag="lgp")
                for m in range(hc):
                    nc.tensor.matmul(
                        lgp[:, :], z2c[:, m:m + 1], ghT[:, m, :],
                        start=(m == 0), stop=(m == hc - 1),
                    )
                lstage = ph2.tile([1, gb * s_len], F32, tag="lstage")
                nc.vector.tensor_copy(lstage[:, :], lgp[:, :])
                nc.sync.dma_start(
                    io["logit_dram"][None, g * gb:(g + 1) * gb, :],
                    lstage[:, :].rearrange("o (b s) -> o b s", b=gb),
                )

            # ---- P3: prall/phall = fT*rowsum(W) + bias (bulk, bf16) ----
            for m in range(hc):
                nc.vector.tensor_scalar(
                    prall[:, m, :, :], fT_all[:, m, :, :],
                    rs4[:, m, 0:1], brc4[:, m:m + 1], ALU.mult, ALU.add,
                )
                nc.vector.tensor_scalar(
                    phall[:, m, :, :], fT_all[:, m, :, :],
                    rs4[:, m, 1:2], bw4[:, m:m + 1], ALU.mult, ALU.add,
                )

        # ============ P4: softmax over S + G broadcast ============
        gbc = smallpool.tile([128, s_len, b_loc], F32, tag="gbc")
        omgbc = smallpool.tile([128, s_len, b_loc], F32, tag="omgbc")
        with tc.tile_pool(name="smax", bufs=1) as sp:
            nc.sync.dma_start(logit[:, :], io["logit_dram"][:, :])
            negmax = sp.tile([b_loc, 1], F32, tag="negmax")
            nc.vector.tensor_reduce(
                negmax[:, :], logit[:, :], mybir.AxisListType.X, ALU.max, negate=True
            )
            esum = sp.tile([b_loc, 1], F32, tag="esum")
            gexp = sp.tile([b_loc, s_len], F32, tag="gexp")
            nc.scalar.activation(
                gexp[:, :], logit[:, :], AF.Exp, bias=negmax[:, :],
                accum_out=esum[:, :],
            )
            inv = sp.tile([b_loc, 1], F32, tag="inv")
            nc.vector.reciprocal(inv[:, :], esum[:, :])
            gmat = sp.tile([b_loc, s_len], F32, tag="gmat")
            nc.vector.tensor_scalar_mul(gmat[:, :], gexp[:, :], inv[:, :])

            # broadcast G to all partitions through a DRAM bounce
            nc.sync.dma_start(io["g_bounce"].rearrange("s b -> b s"), gmat[:, :])
            nc.sync.dma_start(
                gbc[:, :, :],
                io["g_bounce"][None, :, :].to_broadcast([128, s_len, b_loc]),
            )
            nc.vector.tensor_scalar(
                omgbc[:, :, :], gbc[:, :, :], -1.0, 1.0, ALU.mult, ALU.add
            )

        # ============ P5: GRU scan ============
        with (
            tc.tile_pool(name="scw", bufs=1) as scw,
            tc.tile_pool(name="scan_sb", bufs=3) as scp,
            tc.tile_pool(name="scan_ps", bufs=4, space="PSUM") as sps,
            tc.tile_pool(name="out_ps", bufs=1, space="PSUM") as ops,
        ):
            # final-layer weights (loaded while the scan runs)
            nmw = scw.tile([128, 3 * hc * h], F32R, tag="nmw")
            for j in range(3 * hc):
                nc.sync.dma_start(
                    nmw[:, j * h:(j + 1) * h],
                    io["nm_w"][j * 128:(j + 1) * 128, :].bitcast(F32R),
                )

            def wslice(gate, m, c):
                return wcomb[:, c * 2 * h + gate * h + m * 128:
                             c * 2 * h + gate * h + (m + 1) * 128]

            # State as two parts: C_{s-1} = pA + pB with
            #   pA = (1-g_{s-1}) * C_{s-2}   (ready EARLY in step s-1)
            #   pB = g_{s-1} * h_{s-1}       (the late tanh product)
            # The pre-activation matmuls are split by linearity:
            #   W^T C = W^T pA + W^T pB
            # so the pA matmuls prefire during the previous step's tail and
            # only the 32 pB matmuls sit on the serial cycle.
            pA = scp.tile([128, hc, b_loc], BF16, tag="pA")
            pB = scp.tile([128, hc, b_loc], BF16, tag="pB")
            nc.vector.memset(pA[:, :, :], 0.0)
            nc.vector.memset(pB[:, :, :], 0.0)

            # step-0 psum: seeds only (C_{-1} = 0)
            ps = sps.tile([128, 2, hc, b_loc], F32, tag="ps")
            nc.tensor.matmul(
                ps[:, 0, :, :], identb[:, :], prall[:, :, :, 0],
                start=True, stop=False,
            )
            nc.tensor.matmul(
                ps[:, 1, :, :], identb[:, :], bu_bc[:, :, :],
                start=True, stop=True, skip_group_check=True,
            )

            for s in range(s_len):
                last = s == s_len - 1
                # gh-part matmuls (the serial-critical ones); none at s=0
                if s > 0:
                    for m in range(hc):
                        for c in range(hc):
                            nc.tensor.matmul(
                                ps[:, 0, m, :], wslice(0, m, c), pB[:, c, :],
                                start=False, stop=False,
                                skip_group_check=True,
                            )
                rt = scp.tile([128, hc, b_loc], BF16, tag="rt")
                nc.scalar.activation(rt[:, :, :], ps[:, 0, :, :], AF.Sigmoid)
                if s > 0:
                    for m in range(hc):
                        for c in range(hc):
                            nc.tensor.matmul(
                                ps[:, 1, m, :], wslice(1, m, c), pB[:, c, :],
                                start=False,
                                stop=(m == hc - 1 and c == hc - 1),
                                skip_group_check=True,
                            )
                # off-path: C_{s-1} = pA + pB, then pA' = (1-g_s) * C_{s-1}
                ctv = scp.tile([128, hc, b_loc], BF16, tag="ctv")
                nc.vector.tensor_add(ctv[:, :, :], pA[:, :, :], pB[:, :, :])
                pA2 = scp.tile([128, hc, b_loc], BF16, tag="pA")
                nc.vector.tensor_mul(
                    pA2[:, :, :], ctv[:, :, :],
                    omgbc[:, s:s + 1, :].to_broadcast([128, hc, b_loc]),
                )
                # next step's psum: seeds + pA'-part matmuls (prefire in the
                # tail's shadow; they only need pA')
                if not last:
                    ps2 = sps.tile([128, 2, hc, b_loc], F32, tag="ps")
                    nc.tensor.matmul(
                        ps2[:, 0, :, :], identb[:, :], prall[:, :, :, s + 1],
                        start=True, stop=False,
                    )
                    nc.tensor.matmul(
                        ps2[:, 1, :, :], identb[:, :], bu_bc[:, :, :],
                        start=True, stop=False, skip_group_check=True,
                    )
                    for gate in range(2):
                        for m in range(hc):
                            for c in range(hc):
                                nc.tensor.matmul(
                                    ps2[:, gate, m, :], wslice(gate, m, c),
                                    pA2[:, c, :],
                                    start=False, stop=False,
                                    skip_group_check=True,
                                )
                # tail: h = tanh(pre_h[s] + r*(C@U + bu));  pB' = g_s * h
                ut2 = scp.tile([128, hc, b_loc], BF16, tag="ut2")
                nc.vector.tensor_mul(ut2[:, :, :], rt[:, :, :], ps[:, 1, :, :])
                hin = scp.tile([128, hc, b_loc], BF16, tag="hin")
                nc.vector.tensor_tensor(
                    hin[:, :, :], ut2[:, :, :], phall[:, :, :, s], ALU.add
                )
                ht = scp.tile([128, hc, b_loc], BF16, tag="ht")
                nc.scalar.activation(ht[:, :, :], hin[:, :, :], AF.Tanh)
                pB2 = scp.tile([128, hc, b_loc], BF16, tag="pB")
                nc.vector.tensor_mul(
                    pB2[:, :, :], ht[:, :, :],
                    gbc[:, s:s + 1, :].to_broadcast([128, hc, b_loc]),
                )
                pA, pB = pA2, pB2
                if not last:
                    ps = ps2

            # final state C = pA + pB
            ct = scp.tile([128, hc, b_loc], BF16, tag="ctfin")
            nc.vector.tensor_add(ct[:, :, :], pA[:, :, :], pB[:, :, :])

            # ============ P6: next memory ============
            ctf = scp.tile([128, hc, b_loc], F32R, tag="ctf")
            ctstg = scp.tile([128, hc, b_loc], F32, tag="ctstg")
            nc.vector.tensor_copy(ctstg[:, :, :], ct[:, :, :])
            nc.vector.tensor_copy(ctf[:, :, :], ctstg[:, :, :])
            po = ops.tile([b_loc, h], F32, tag="po")
            chunks = [mT, ctf, qT]
            for part in range(3):
                src = chunks[part]
                for c in range(hc):
                    j = part * hc + c
                    nc.tensor.matmul(
                        po[:, :],
                        src[:, c, :],
                        nmw[:, j * h:(j + 1) * h],
                        start=(j == 0), stop=False,
                    )
            nc.tensor.matmul(
                po[:, :], ones_row[:, :], nmb_row[:, :], start=False, stop=True
            )
            out_sb = scp.tile([b_loc, h], F32, tag="out_sb")
            nc.scalar.activation(out_sb[:, :], po[:, :], AF.Relu)
            nc.sync.dma_start(io["out"][:, 0, :], out_sb[:, :])


_NC_CACHE = {}


def _run(inputs, **spmd_kwargs):
    if "full" not in _NC_CACHE:
        _NC_CACHE["full"] = build_nc()
    nc = _NC_CACHE["full"]

    names = ["facts", "questions", "prevM", "z1_w", "z1_b", "z2_w",
             "Wr", "br", "Ur", "bur", "W", "bw", "U", "bu", "nm_w", "nm_b"]
    sharded = {"facts", "questions", "prevM"}
    in_maps = []
    for i in range(N_CORES):
        m = {}
        for n in names:
            v = np.asarray(inputs[n], dtype=np.float32)
            if n in sharded:
                v = v[i * B_LOC:(i + 1) * B_LOC]
            m[n] = np.ascontiguousarray(v)
        in_maps.append(m)

    res = run_bass_kernel_spmd(nc, in_maps, list(range(N_CORES)), **spmd_kwargs)
    out = np.concatenate(
        [res.results[i]["out"] for i in range(N_CORES)], axis=0
    ).astype(np.float32)
    return out, res


def kernel(**inputs):
    return _run(inputs)[0]


# revision 18
# speedup vs baseline: 1.1525x; 1.1525x over previous
"""Trainium2 Bass kernel for EpisodicMemory (DMN episodic memory module).

Full shapes: facts (128,256,512), questions/prevM (128,1,512), output (128,1,512).
Sharding: data-parallel over batch, 16 batches per core x 8 cores, weights
replicated. Activations kept feature-on-partition ("transposed") so matmuls
contract over the partition dim and pointwise ops run 128 lanes wide.

Per-core pipeline (v2 — bf16 everywhere on the PE so FWL fast-weight-load
kicks in; the fp32 path disables FWL and double-passes every matmul):
  P1  facts -> bf16 -> PE transposes -> fT_all (128, hc, b, s) resident
  P2  interaction features zT (bf16) -> z1 MLP (tanh) -> z2 logits
  P3  prall/phall = fT*rowsum(W) + bias  (bulk precompute, bf16)
  P4  softmax over S -> G broadcast gbc + (1-G) omgbc
  P5  GRU scan, 256 steps: PSUM seeded with prall[s]/bu via identity
      matmuls, bf16 weight-stationary matmuls, short DVE/ACT tail
  P6  next_mem = relu([prevM C q] @ nm_w + nm_b)
"""

from contextlib import ExitStack

import numpy as np

import concourse.bass as bass
import concourse.tile as tile
from concourse import bacc, masks, mybir
from concourse.bass_utils import run_bass_kernel_spmd

F32 = mybir.dt.float32
F32R = mybir.dt.float32r
BF16 = mybir.dt.bfloat16
AF = mybir.ActivationFunctionType
ALU = mybir.AluOpType

B, S, H = 128, 256, 512
N_CORES = 8
B_LOC = B // N_CORES  # 16


def build_nc(b_loc=B_LOC, s_len=S):
    """Build the per-core Bass program (SPMD: same program, sharded data)."""
    h = H
    nc = bacc.Bacc(
        "TRN2", target_bir_lowering=False, debug=False, num_devices=N_CORES
    )

    io = {}
    io["facts"] = nc.dram_tensor("facts", [b_loc, s_len, h], F32, kind="ExternalInput")
    io["questions"] = nc.dram_tensor("questions", [b_loc, 1, h], F32, kind="ExternalInput")
    io["prevM"] = nc.dram_tensor("prevM", [b_loc, 1, h], F32, kind="ExternalInput")
    io["z1_w"] = nc.dram_tensor("z1_w", [4 * h, h], F32, kind="ExternalInput")
    io["z1_b"] = nc.dram_tensor("z1_b", [h], F32, kind="ExternalInput")
    io["z2_w"] = nc.dram_tensor("z2_w", [h, 1], F32, kind="ExternalInput")
    for nm in ["Wr", "Ur", "W", "U"]:
        io[nm] = nc.dram_tensor(nm, [h, h], F32, kind="ExternalInput")
    for nm in ["br", "bur", "bw", "bu"]:
        io[nm] = nc.dram_tensor(nm, [h], F32, kind="ExternalInput")
    io["nm_w"] = nc.dram_tensor("nm_w", [3 * h, h], F32, kind="ExternalInput")
    io["nm_b"] = nc.dram_tensor("nm_b", [h], F32, kind="ExternalInput")
    io["out"] = nc.dram_tensor("out", [b_loc, 1, h], F32, kind="ExternalOutput")
    io["g_bounce"] = nc.dram_tensor("g_bounce", [s_len, b_loc], F32)
    io["logit_dram"] = nc.dram_tensor("logit_dram", [b_loc, s_len], F32)

    with tile.TileContext(nc) as tc:
        _body(tc, io, b_loc, s_len, h)
    nc.compile()
    return nc


def _body(tc, io, b_loc, s_len, h):
    nc = tc.nc
    hc = h // 128          # 4 h-chunks
    zc = 4 * hc            # 16 chunks of the 4H interaction dim
    gb = 2                 # batches per group for the z1 MLP
    ng = b_loc // gb
    sc_ = s_len // 128

    facts, questions, prevM = io["facts"], io["questions"], io["prevM"]

    with ExitStack() as ctx:
        # ---------------- resident pools ----------------
        wpool = ctx.enter_context(tc.tile_pool(name="wres", bufs=1))
        prepool = ctx.enter_context(tc.tile_pool(name="prepool", bufs=1))
        smallpool = ctx.enter_context(tc.tile_pool(name="small", bufs=1))

        # scan gate weights [Ur | U] in bf16: k-chunk c at cols [c*2h, (c+1)*2h)
        wcomb = wpool.tile([128, hc * 2 * h], BF16, tag="wcomb")
        wstg_pool = ctx.enter_context(tc.tile_pool(name="wstg", bufs=2))
        for c in range(hc):
            for gi, wn in enumerate(["Ur", "U"]):
                stg = wstg_pool.tile([128, h], F32, tag="wstg")
                nc.sync.dma_start(stg[:, :], io[wn][c * 128:(c + 1) * 128, :])
                nc.vector.tensor_copy(
                    wcomb[:, c * 2 * h + gi * h: c * 2 * h + (gi + 1) * h],
                    stg[:, :],
                )

        # small constants: (128, hc) with col = h-chunk
        def load_cvec(nm):
            t = smallpool.tile([128, hc], F32, tag=f"cv_{nm}")
            nc.sync.dma_start(t[:, :], io[nm].rearrange("(c p) -> p c", p=128))
            return t

        z1b4 = load_cvec("z1_b")
        br4 = load_cvec("br")
        bur4 = load_cvec("bur")
        bw4 = load_cvec("bw")
        bu4 = load_cvec("bu")
        z2c = smallpool.tile([128, hc], BF16, tag="z2c")
        z2stg = smallpool.tile([128, hc], F32, tag="z2stg")
        nc.sync.dma_start(
            z2stg[:, :], io["z2_w"].rearrange("(c p) o -> p (c o)", p=128)
        )
        nc.vector.tensor_copy(z2c[:, :], z2stg[:, :])
        brc4 = smallpool.tile([128, hc], F32, tag="brc4")  # br + bur
        nc.vector.tensor_copy(brc4[:, :], br4[:, :])
        nc.vector.tensor_add(brc4[:, :], brc4[:, :], bur4[:, :])

        ones_pb = smallpool.tile([128, b_loc], F32, tag="ones_pb")
        nc.vector.memset(ones_pb[:, :], 1.0)
        # psu PSUM seed: bu broadcast along batches, (128, hc, b_loc)
        bu_bc = smallpool.tile([128, hc, b_loc], BF16, tag="bu_bc")
        for c in range(hc):
            nc.vector.tensor_scalar_mul(bu_bc[:, c, :], ones_pb[:, :], bu4[:, c:c + 1])

        # questions / prevM transposed: (128, hc, b_loc)
        qT = smallpool.tile([128, hc, b_loc], F32R, tag="qT")
        mT = smallpool.tile([128, hc, b_loc], F32R, tag="mT")
        for bi in range(b_loc):
            nc.sync.dma_start(
                qT[:, :, bi],
                questions[bi, 0, :].rearrange("(c p) -> p c", p=128).bitcast(F32R),
            )
            nc.sync.dma_start(
                mT[:, :, bi],
                prevM[bi, 0, :].rearrange("(c p) -> p c", p=128).bitcast(F32R),
            )
        nqT = smallpool.tile([128, hc, b_loc], F32, tag="nqT")
        nmT = smallpool.tile([128, hc, b_loc], F32, tag="nmT")
        nc.vector.tensor_scalar_mul(nqT[:, :, :], qT[:, :, :].bitcast(F32), -1.0)
        nc.vector.tensor_scalar_mul(nmT[:, :, :], mT[:, :, :].bitcast(F32), -1.0)

        ones_row = smallpool.tile([1, b_loc], F32R, tag="ones_row")
        ones_stg = smallpool.tile([1, b_loc], F32, tag="ones_stg")
        nc.vector.memset(ones_stg[:, :], 1.0)
        nc.vector.tensor_copy(ones_row[:, :], ones_stg[:, :])
        nmb_row = smallpool.tile([1, h], F32R, tag="nmb_row")
        nc.sync.dma_start(nmb_row[:, :], io["nm_b"][None, :].bitcast(F32R))

        identb = smallpool.tile([128, 128], BF16, tag="identb")
        masks.make_identity(nc, identb[:, :])

        # facts transposed, resident: (128, hc, b_loc, s_len) bf16
        fT_all = prepool.tile([128, hc, b_loc, s_len], BF16, tag="fT_all")
        # pre-activations incl. biases, resident: (128, hc, b_loc, s_len) bf16
        prall = prepool.tile([128, hc, b_loc, s_len], BF16, tag="prall")
        phall = prepool.tile([128, hc, b_loc, s_len], BF16, tag="phall")
        logit = smallpool.tile([b_loc, s_len], F32, tag="logit")

        # row-sums of Wr / W (the reference's einsum "bsh,hk->bsh"
        # multiplies facts elementwise by these row-sums)
        rs4 = smallpool.tile([128, hc, 2], F32, tag="rs4")

        # ============ P1: load facts, cast bf16, PE-transpose ============
        with (
            tc.tile_pool(name="ph", bufs=3) as ph,
            tc.tile_pool(name="tps", bufs=2, space="PSUM") as tps,
        ):
            for gate, wname in enumerate(["Wr", "W"]):
                for c in range(hc):
                    wstg = ph.tile([128, h], F32, tag="rstg")
                    nc.sync.dma_start(
                        wstg[:, :], io[wname][c * 128:(c + 1) * 128, :]
                    )
                    nc.vector.tensor_reduce(
                        rs4[:, c, gate:gate + 1], wstg[:, :],
                        mybir.AxisListType.X, ALU.add,
                    )

            for bi in range(b_loc):
                fnat = ph.tile([128, sc_, h], F32, tag="fnat")
                nc.sync.dma_start(
                    fnat[:, :, :],
                    facts[bi].rearrange("(c p) h -> p c h", p=128),
                )
                fnb = ph.tile([128, sc_, h], BF16, tag="fnb")
                nc.scalar.copy(fnb[:, :, :], fnat[:, :, :])
                for sh in range(sc_):
                    tp = tps.tile([128, hc, 128], BF16, tag="tpsum")
                    for c in range(hc):
                        nc.tensor.transpose(
                            tp[:, c, :], fnb[:, sh, c * 128:(c + 1) * 128],
                            identb[:, :],
                        )
                    nc.vector.tensor_copy(
                        fT_all[:, :, bi, sh * 128:(sh + 1) * 128], tp[:, :, :]
                    )

        # ============ P2: interaction features + z1 MLP + z2 logits ====
        with (
            tc.tile_pool(name="z1wp", bufs=1) as z1wp,
            tc.tile_pool(name="ph2", bufs=2) as ph2,
            tc.tile_pool(name="zpool", bufs=3) as zp,
            tc.tile_pool(name="ghps", bufs=1, space="PSUM") as ghps,
            tc.tile_pool(name="lgps", bufs=1, space="PSUM") as lgps,
        ):
            # z1 weights in bf16 (staged through f32)
            z1w = z1wp.tile([128, zc * h], BF16, tag="z1w")
            for k in range(zc):
                stg = ph2.tile([128, h], F32, tag="z1stage")
                nc.sync.dma_start(stg[:, :], io["z1_w"][k * 128:(k + 1) * 128, :])
                nc.vector.tensor_copy(z1w[:, k * h:(k + 1) * h], stg[:, :])

            for g in range(ng):
                ghp = [ghps.tile([128, gb * s_len], F32, name=f"ghp{m}",
                                 tag=f"ghp{m}") for m in range(hc)]
                for k in range(zc):
                    zk = zp.tile([128, gb * s_len], BF16, tag="zk")
                    kind, c = divmod(k, hc)  # 0:f*q 1:f*m 2:|f-q| 3:|f-m|
                    for bp in range(gb):
                        bi = g * gb + bp
                        dst = zk[:, bp * s_len:(bp + 1) * s_len]
                        src = fT_all[:, c, bi, :]
                        if kind == 0:
                            nc.vector.tensor_scalar_mul(dst, src, qT[:, c, bi:bi + 1].bitcast(F32))
                        elif kind == 1:
                            nc.vector.tensor_scalar_mul(dst, src, mT[:, c, bi:bi + 1].bitcast(F32))
                        elif kind == 2:
                            nc.scalar.activation(dst, src, AF.Abs,
                                                 bias=nqT[:, c, bi:bi + 1])
                        else:
                            nc.scalar.activation(dst, src, AF.Abs,
                                                 bias=nmT[:, c, bi:bi + 1])
                    for m in range(hc):
                        nc.tensor.matmul(
                            ghp[m][:, :],
                            z1w[:, k * h + m * 128: k * h + (m + 1) * 128],
                            zk[:, :],
                            start=(k == 0),
                            stop=(k == zc - 1),
                        )
                ghT = ph2.tile([128, hc, gb * s_len], BF16, tag="ghT")
                for m in range(hc):
                    nc.scalar.activation(
                        ghT[:, m, :], ghp[m][:, :], AF.Tanh,
                        bias=z1b4[:, m:m + 1],
                    )
                lgp = lgps.tile([1, gb * s_len], F32, tag="lgp")
                for m in range(hc):
                    nc.tensor.matmul(
                        lgp[:, :], z2c[:, m:m + 1], ghT[:, m, :],
                        start=(m == 0), stop=(m == hc - 1),
                    )
                lstage = ph2.tile([1, gb * s_len], F32, tag="lstage")
                nc.vector.tensor_copy(lstage[:, :], lgp[:, :])
                nc.sync.dma_start(
                    io["logit_dram"][None, g * gb:(g + 1) * gb, :],
                    lstage[:, :].rearrange("o (b s) -> o b s", b=gb),
                )

            # ---- P3: prall/phall = fT*rowsum(W) + bias (bulk, bf16) ----
            for m in range(hc):
                nc.vector.tensor_scalar(
                    prall[:, m, :, :], fT_all[:, m, :, :],
                    rs4[:, m, 0:1], brc4[:, m:m + 1], ALU.mult, ALU.add,
                )
                nc.vector.tensor_scalar(
                    phall[:, m, :, :], fT_all[:, m, :, :],
                    rs4[:, m, 1:2], bw4[:, m:m + 1], ALU.mult, ALU.add,
                )

        # ============ P4: softmax over S + G broadcast ============
        gbc = smallpool.tile([128, s_len, b_loc], F32, tag="gbc")
        omgbc = smallpool.tile([128, s_len, b_loc], F32, tag="omgbc")
        with tc.tile_pool(name="smax", bufs=1) as sp:
            nc.sync.dma_start(logit[:, :], io["logit_dram"][:, :])
            negmax = sp.tile([b_loc, 1], F32, tag="negmax")
            nc.vector.tensor_reduce(
                negmax[:, :], logit[:, :], mybir.AxisListType.X, ALU.max, negate=True
            )
            esum = sp.tile([b_loc, 1], F32, tag="esum")
            gexp = sp.tile([b_loc, s_len], F32, tag="gexp")
            nc.scalar.activation(
                gexp[:, :], logit[:, :], AF.Exp, bias=negmax[:, :],
                accum_out=esum[:, :],
            )
            inv = sp.tile([b_loc, 1], F32, tag="inv")
            nc.vector.reciprocal(inv[:, :], esum[:, :])
            gmat = sp.tile([b_loc, s_len], F32, tag="gmat")
            nc.vector.tensor_scalar_mul(gmat[:, :], gexp[:, :], inv[:, :])

            # broadcast G to all partitions through a DRAM bounce
            nc.sync.dma_start(io["g_bounce"].rearrange("s b -> b s"), gmat[:, :])
            nc.sync.dma_start(
                gbc[:, :, :],
                io["g_bounce"][None, :, :].to_broadcast([128, s_len, b_loc]),
            )
            nc.vector.tensor_scalar(
                omgbc[:, :, :], gbc[:, :, :], -1.0, 1.0, ALU.mult, ALU.add
            )

        # ============ P5: GRU scan ============
        with (
            tc.tile_pool(name="scw", bufs=1) as scw,
            tc.tile_pool(name="scan_sb", bufs=3) as scp,
            tc.tile_pool(name="scan_ps", bufs=3, space="PSUM") as sps,
            tc.tile_pool(name="out_ps", bufs=1, space="PSUM") as ops,
        ):
            # final-layer weights (loaded while the scan runs)
            nmw = scw.tile([128, 3 * hc * h], F32R, tag="nmw")
            for j in range(3 * hc):
                nc.sync.dma_start(
                    nmw[:, j * h:(j + 1) * h],
                    io["nm_w"][j * 128:(j + 1) * 128, :].bitcast(F32R),
                )

            def wslice(gate, m, c):
                return wcomb[:, c * 2 * h + gate * h + m * 128:
                             c * 2 * h + gate * h + (m + 1) * 128]

            # State as two parts: C_{s-1} = pA + pB with
            #   pA = (1-g_{s-1}) * C_{s-2}   (ready EARLY in step s-1)
            #   pB = g_{s-1} * h_{s-1}       (the late tanh product)
            # The pre-activation matmuls are split by linearity:
            #   W^T C = W^T pA + W^T pB
            # so the pA matmuls (+ next step's seeds) prefire during the
            # tanh tail and only the 32 pB matmuls sit on the serial cycle.
            pA = scp.tile([128, hc, b_loc], BF16, tag="pA")
            pB = scp.tile([128, hc, b_loc], BF16, tag="pB")
            nc.vector.memset(pA[:, :, :], 0.0)
            nc.vector.memset(pB[:, :, :], 0.0)

            # step-0 psums: seeds only (C_{-1} = 0)
            psr = sps.tile([128, hc, b_loc], F32, tag="psr")
            psu = sps.tile([128, hc, b_loc], F32, tag="psu")
            nc.tensor.matmul(
                psr[:, :, :], identb[:, :], prall[:, :, :, 0],
                start=True, stop=True,
            )
            nc.tensor.matmul(
                psu[:, :, :], identb[:, :], bu_bc[:, :, :],
                start=True, stop=True,
            )

            for s in range(s_len):
                last = s == s_len - 1
                # pB-part matmuls (the serial-critical ones); none at s=0
                if s > 0:
                    for m in range(hc):
                        for c in range(hc):
                            nc.tensor.matmul(
                                psr[:, m, :], wslice(0, m, c), pB[:, c, :],
                                start=False,
                                stop=(m == hc - 1 and c == hc - 1),
                            )
                rt = scp.tile([128, hc, b_loc], BF16, tag="rt")
                nc.scalar.activation(rt[:, :, :], psr[:, :, :], AF.Sigmoid)
                if s > 0:
                    for m in range(hc):
                        for c in range(hc):
                            nc.tensor.matmul(
                                psu[:, m, :], wslice(1, m, c), pB[:, c, :],
                                start=False,
                                stop=(m == hc - 1 and c == hc - 1),
                            )
                # off-path: C_{s-1} = pA + pB, then pA' = (1-g_s) * C_{s-1}
                ctv = scp.tile([128, hc, b_loc], BF16, tag="ctv")
                nc.vector.tensor_add(ctv[:, :, :], pA[:, :, :], pB[:, :, :])
                pA2 = scp.tile([128, hc, b_loc], BF16, tag="pA")
                nc.vector.tensor_mul(
                    pA2[:, :, :], ctv[:, :, :],
                    omgbc[:, s:s + 1, :].to_broadcast([128, hc, b_loc]),
                )
                # tail part 1
                ut2 = scp.tile([128, hc, b_loc], BF16, tag="ut2")
                nc.vector.tensor_mul(ut2[:, :, :], rt[:, :, :], psu[:, :, :])
                hin = scp.tile([128, hc, b_loc], BF16, tag="hin")
                nc.vector.tensor_tensor(
                    hin[:, :, :], ut2[:, :, :], phall[:, :, :, s], ALU.add
                )
                # next step's psums: seeds + pA'-part matmuls, emitted here so
                # they fill the PE during the tanh tail (they need only pA')
                if not last:
                    psr2 = sps.tile([128, hc, b_loc], F32, tag="psr")
                    psu2 = sps.tile([128, hc, b_loc], F32, tag="psu")
                    nc.tensor.matmul(
                        psr2[:, :, :], identb[:, :], prall[:, :, :, s + 1],
                        start=True, stop=False,
                    )
                    nc.tensor.matmul(
                        psu2[:, :, :], identb[:, :], bu_bc[:, :, :],
                        start=True, stop=False,
                    )
                    for gate in range(2):
                        ps2 = psr2 if gate == 0 else psu2
                        for m in range(hc):
                            for c in range(hc):
                                nc.tensor.matmul(
                                    ps2[:, m, :], wslice(gate, m, c),
                                    pA2[:, c, :],
                                    start=False, stop=False,
                                )
                # tail part 2: h = tanh(...); pB' = g_s * h
                ht = scp.tile([128, hc, b_loc], BF16, tag="ht")
                nc.scalar.activation(ht[:, :, :], hin[:, :, :], AF.Tanh)
                pB2 = scp.tile([128, hc, b_loc], BF16, tag="pB")
                nc.vector.tensor_mul(
                    pB2[:, :, :], ht[:, :, :],
                    gbc[:, s:s + 1, :].to_broadcast([128, hc, b_loc]),
                )
                pA, pB = pA2, pB2
                if not last:
                    psr, psu = psr2, psu2

            # final state C = pA + pB
            ct = scp.tile([128, hc, b_loc], BF16, tag="ctfin")
            nc.vector.tensor_add(ct[:, :, :], pA[:, :, :], pB[:, :, :])

            # ============ P6: next memory ============
            ctf = scp.tile([128, hc, b_loc], F32R, tag="ctf")
            ctstg = scp.tile([128, hc, b_loc], F32, tag="ctstg")
            nc.vector.tensor_copy(ctstg[:, :, :], ct[:, :, :])
            nc.vector.tensor_copy(ctf[:, :, :], ctstg[:, :, :])
            po = ops.tile([b_loc, h], F32, tag="po")
            chunks = [mT, ctf, qT]
            for part in range(3):
                src = chunks[part]
                for c in range(hc):
                    j = part * hc + c
                    nc.tensor.matmul(
                        po[:, :],
                        src[:, c, :],
                        nmw[:, j * h:(j + 1) * h],
                        start=(j == 0), stop=False,
                    )
            nc.tensor.matmul(
                po[:, :], ones_row[:, :], nmb_row[:, :], start=False, stop=True
            )
            out_sb = scp.tile([b_loc, h], F32, tag="out_sb")
            nc.scalar.activation(out_sb[:, :], po[:, :], AF.Relu)
            nc.sync.dma_start(io["out"][:, 0, :], out_sb[:, :])


_NC_CACHE = {}


def _run(inputs, **spmd_kwargs):
    if "full" not in _NC_CACHE:
        _NC_CACHE["full"] = build_nc()
    nc = _NC_CACHE["full"]

    names = ["facts", "questions", "prevM", "z1_w", "z1_b", "z2_w",
             "Wr", "br", "Ur", "bur", "W", "bw", "U", "bu", "nm_w", "nm_b"]
    sharded = {"facts", "questions", "prevM"}
    in_maps = []
    for i in range(N_CORES):
        m = {}
        for n in names:
            v = np.asarray(inputs[n], dtype=np.float32)
            if n in sharded:
                v = v[i * B_LOC:(i + 1) * B_LOC]
            m[n] = np.ascontiguousarray(v)
        in_maps.append(m)

    res = run_bass_kernel_spmd(nc, in_maps, list(range(N_CORES)), **spmd_kwargs)
    out = np.concatenate(
        [res.results[i]["out"] for i in range(N_CORES)], axis=0
    ).astype(np.float32)
    return out, res


def kernel(**inputs):
    return _run(inputs)[0]
